# revision 38
# baseline (speedup 1.0000x reference)
"""Entropy-gated multi-head attention on 8 Trainium2 NeuronCores.

Sharding: core c = b*4 + g handles batch b (of 2) and head-group g (4 of the
16 heads).  Tokens with gate==0 pass x through untouched and contribute
exactly zero k/v (zero biases), so the device only processes the compacted
active tokens (~half), with the softmax denominator corrected by the count
of inactive tokens: each inactive/padded key contributes exp(0)=1 to the
softmax sum and nothing to the numerator (v=0).

v2 pipeline (ACT-saturating):
  - all inputs fp8, host pre-shuffled to [128, kt, cols] so every input DMA
    is 128 large descriptors; issued across 4 sequencer queues.
  - QT/KT projections: fp8 DoubleRow over k-tile pairs; psum halves copied
    (Pool engine) into the [128, 2, cols] DR-32 score layout (host-permuted
    W columns).
  - scores: fp8 DR-32 per (head, key-tile): lhsT kt[32h:32h+32, 2, 128],
    62ns/tile measured (4 concurrent PE row-tiles at positions 0/32/64/96).
  - exp on ACT in 3-key-tile groups [128, 3, qn] (two rotating 3-bank psum
    buffers) -> ACT is the bottleneck engine and stays ~saturated.
  - OT = [V|1]^T PT accumulated per head over key-tile DR pairs (+1 single).
  - softmax denom: Z row + CADD (Pool) -> reciprocal (DVE) -> PE K=1 matmul
    broadcast into psum -> one STT per head builds fp8 osb (x16 scale kept
    in range); no DRAM round-trips.
  - Y: fp8 DR over the head-pair planes, psum DMA'd straight to DRAM f32;
    host divides by 256 (= WSCALE^2), sums the 4 group partials, scatters.
"""

import heapq
import math
from contextlib import ExitStack

import numpy as np
import ml_dtypes

import concourse.bass as bass
import concourse.mybir as mybir
from concourse import bacc
import concourse.tile as tile
from concourse.bass_utils import run_bass_kernel_spmd

B, S, D = 2, 2048, 1024
H, DH = 16, 64
NCORES = 8
GROUPS = NCORES // B          # head-groups per batch = 4
HC = H // GROUPS              # heads per core = 4
DC = HC * DH                  # head-group width = 256

WSCALE = 16.0                 # host scale on Wq/Wk/Wv/Wo fp8 weights
LN16 = float(np.log(16.0))

f32 = mybir.dt.float32
bf16 = mybir.dt.bfloat16
fp8 = mybir.dt.float8e4
DR = mybir.MatmulPerfMode.DoubleRow
npf8 = ml_dtypes.float8_e4m3


def _qchunks(saq):
    out = []
    o = 0
    while o < saq:
        out.append((o, min(512, saq - o)))
        o += 512
    return out


def _build(SAQ: int, SAK: int) -> bass.Bass:
    nkt = D // 128            # 8 contraction tiles for projections
    nst = SAK // 128          # key tiles (kt cols are 128-padded)
    SAQP = (SAQ + 15) // 16 * 16   # qt free stride: DR subtile stride % 16
    qch = _qchunks(SAQ)
    kch = _qchunks(SAK)
    # pt = exp(s)/16; each key (incl. padding) contributes exp(0)/16 -> Z is
    # corrected by the tokens not present on the device at all:
    CADD = float(S - SAK) / 16.0
    ESCALE = 0.125 / (WSCALE * WSCALE)

    nc = bacc.Bacc()
    xT_d = nc.dram_tensor("xT", [128, nkt, SAK], fp8, kind="ExternalInput")
    wq_d = nc.dram_tensor("wq", [128, nkt, DC], fp8, kind="ExternalInput")
    wk_d = nc.dram_tensor("wk", [128, nkt, DC], fp8, kind="ExternalInput")
    wv_d = nc.dram_tensor("wv", [128, nkt, DC], fp8, kind="ExternalInput")
    wo_d = nc.dram_tensor("wo", [128, 2, D], fp8, kind="ExternalInput")
    y_d = nc.dram_tensor("y", [SAQ, D], bf16, kind="ExternalOutput")
    # narrow-tail path: per-head unnormalized Y + raw Z rows; the softmax
    # division happens on the host (removes the whole on-device r chain
    # from the critical tail)
    yt_d = nc.dram_tensor("yt", [4, 64, D], bf16, kind="ExternalOutput")
    zt_d = nc.dram_tensor("zt", [1, 4, 64], f32, kind="ExternalOutput")

    with tile.TileContext(nc) as tc, ExitStack() as ctx:
        singles = ctx.enter_context(tc.tile_pool(name="singles", bufs=1))
        pt_pool = ctx.enter_context(tc.tile_pool(name="pt", bufs=8))
        osb_pool = ctx.enter_context(tc.tile_pool(name="osb", bufs=2))
        zscr = ctx.enter_context(tc.tile_pool(name="zscr", bufs=12,
                                              space="DRAM"))
        yo_pool = ctx.enter_context(tc.tile_pool(name="yo", bufs=3))
        z_pool = ctx.enter_context(tc.tile_pool(name="z", bufs=4))
        # PSUM (8 banks): scores own a 2x2-bank ring paced only by ACT;
        # projections and Y rotate through a separate 1-bank aux pool so
        # their slow DVE-paced releases never stall the score pipeline
        stp = ctx.enter_context(tc.tile_pool(name="stp", bufs=4, space="PSUM"))
        otp = ctx.enter_context(tc.tile_pool(name="otp", bufs=4, space="PSUM"))

        # ---- persistent SBUF ----
        xt = singles.tile([128, nkt, SAK], fp8)
        wq_sb = singles.tile([128, nkt, DC], fp8)
        wk_sb = singles.tile([128, nkt, DC], fp8)
        wv_sb = singles.tile([128, nkt, DC], fp8)
        wo_sb = singles.tile([128, 2, D], fp8)
        # DR-32 score layout: head h on partitions 32h..32h+32; dim d of head
        # h at (p = 32h + d%32, j = d//32); W cols host-permuted to match.
        qt = singles.tile([128, 2, SAQP], fp8)
        kt = singles.tile([128, 2, SAK], fp8)
        # 68 cols (V | ones | pad); dual-fp8 LDWEIGHTS subtile stride must be
        # a multiple of 16 -> cols % 4 == 0
        v_aug = singles.tile([128, nst, HC, 68], fp8)
        ebias = singles.tile([128, 1], f32)

        # ---- input DMAs: 4 issuing queues, xT split so QT can start on
        # k-tile pair 0 while later pairs stream in
        nc.sync.dma_start(xt[:, 0:2, :], xT_d[:, 0:2, :])
        nc.scalar.dma_start(wq_sb[:, :, :], wq_d[:, :, :])
        nc.gpsimd.dma_start(wk_sb[:, :, :], wk_d[:, :, :])
        nc.sync.dma_start(xt[:, 2:4, :], xT_d[:, 2:4, :])
        nc.scalar.dma_start(xt[:, 4:6, :], xT_d[:, 4:6, :])
        nc.gpsimd.dma_start(xt[:, 6:8, :], xT_d[:, 6:8, :])
        nc.sync.dma_start(wv_sb[:, :, :], wv_d[:, :, :])
        nc.scalar.dma_start(wo_sb[:, :, :], wo_d[:, :, :])

        nc.gpsimd.memset(v_aug[:, :, :, 64:65], 1.0)
        nc.gpsimd.memset(v_aug[:, :, :, 65:68], 0.0)
        nc.gpsimd.memset(ebias, -LN16)

        # ---- projections ----
        def proj_qk(dst, w_sb, m, q0, qn):
            ps = stp.tile([128, 512], f32, tag="st", name="pqk")
            for t2 in range(nkt // 2):
                nc.tensor.matmul(
                    ps[:, :qn],
                    w_sb[:, 2 * t2:2 * t2 + 2, m * 128:(m + 1) * 128],
                    xt[:, 2 * t2:2 * t2 + 2, q0:q0 + qn],
                    start=(t2 == 0), stop=(t2 == nkt // 2 - 1),
                    perf_mode=DR)
            # psum halves -> DR-32 layout (cross-partition-base copies)
            nc.vector.tensor_copy(dst[64 * m:64 * m + 64, 0, q0:q0 + qn],
                                  ps[0:64, :qn])
            nc.vector.tensor_copy(dst[64 * m:64 * m + 64, 1, q0:q0 + qn],
                                  ps[64:128, :qn])

        v_done = set()

        def proj_v(s):
            v_done.add(s)
            ps = stp.tile([128, 512], f32, tag="st", name="pv")
            for t2 in range(nkt // 2):
                nc.tensor.matmul(
                    ps[:, :DC],
                    xt[:, 2 * t2:2 * t2 + 2, s * 128:(s + 1) * 128],
                    wv_sb[:, 2 * t2:2 * t2 + 2, :],
                    start=(t2 == 0), stop=(t2 == nkt // 2 - 1),
                    perf_mode=DR)
            nc.vector.tensor_copy(
                v_aug[:, s, :, 0:64],
                ps[:, :DC].rearrange("p (h d) -> p h d", h=HC))

        # natural order: the narrow tail chunk runs LAST — its endgame
        # (chain + 4 tiny STTs + 1 small y) is far cheaper than a wide
        # chunk's, and wide chunks have enough slots to hide the previous
        # chunk's chain/STT latency behind their own score/exp stream
        qord0 = list(qch)
        # upfront: only what the very first exp needs (qt chunk0 m0 + kt k0
        # m0); everything else is pulled on demand so ACT starts early
        proj_qk(qt, wq_sb, 0, qord0[0][0], qord0[0][1])
        proj_qk(kt, wk_sb, 0, kch[0][0], kch[0][1])
        qk_done = {("q", 0, qord0[0][0]), ("k", 0, 0)}

        # lazily drained PE work.  NOTE: emission order is semantic order in
        # Tile — a consumer emitted before its producer reads stale SBUF, so
        # every score emission explicitly pulls its qt/kt producers first.
        vq = list(range(nst))
        aux = [(("q", 1, qord0[0][0]),
                lambda: proj_qk(qt, wq_sb, 1, qord0[0][0], qord0[0][1]))]
        for kc, (k0, kn) in enumerate(kch):
            if kc > 0:
                aux.append((("k", 0, kc),
                            lambda a=k0, b=kn: proj_qk(kt, wk_sb, 0, a, b)))
            aux.append((("k", 1, kc),
                        lambda a=k0, b=kn: proj_qk(kt, wk_sb, 1, a, b)))
        for (q0, qn) in qord0[1:]:
            aux.append((("q", 0, q0),
                        lambda a=q0, b=qn: proj_qk(qt, wq_sb, 0, a, b)))
            aux.append((("q", 1, q0),
                        lambda a=q0, b=qn: proj_qk(qt, wq_sb, 1, a, b)))

        def ensure_v(s):
            while s not in v_done:
                proj_v(vq.pop(0))

        def drain(k):
            for _ in range(min(k, len(aux))):
                key, fn = aux.pop(0)
                qk_done.add(key)
                fn()

        def ensure_q(m, q0):
            while ("q", m, q0) not in qk_done:
                drain(1)

        def ensure_km(m, s_hi):
            # kt chunks are 512 cols = 4 key tiles each
            for kc in range((s_hi // 4) + 1):
                while ("k", m, kc) not in qk_done:
                    drain(1)

        # ---- attention: lag-scheduled emission ----
        # Every engine queue is in-order, so an instruction emitted before
        # its cross-engine producers have RUN stalls everything behind it
        # in its queue.  Scores+exps are the clock; all other work (OT, the
        # softmax chain, osb, Y, projections) is deferred via a small event
        # list and emitted a few score-slots after its producers.
        sched = []
        seqn = [0]
        slot_i = [0]
        zq_rot = [0]
        ZQS = [nc.sync, nc.scalar, nc.gpsimd]

        def after(lag, fn):
            seqn[0] += 1
            heapq.heappush(sched, (slot_i[0] + lag, seqn[0], fn))

        def run_due(flush=False):
            while sched and (flush or sched[0][0] <= slot_i[0]):
                heapq.heappop(sched)[2]()
                if flush:
                    slot_i[0] += 1

        def tick():
            slot_i[0] += 1
            run_due()

        y_osb = {}

        def y_job(q0, qn, jt, d0, act_copy=False):
            qtn = min(128, qn - jt * 128)
            osb = y_osb[(q0, qn)]
            yo = yo_pool.tile([128, 512], bf16, tag="yo", name="yo")
            yps = stp.tile([128, 512], f32, tag="st", name="yps")
            nc.tensor.matmul(
                yps[:qtn, :512],
                osb[:, :, jt * 128:jt * 128 + qtn],
                wo_sb[:, :, d0:d0 + 512],
                start=True, stop=True, perf_mode=DR)
            if act_copy:  # endgame: ACT is idle, DVE is the serial tail
                nc.scalar.copy(yo[:qtn, :], yps[:qtn, :512])
            else:
                nc.vector.tensor_copy(yo[:qtn, :], yps[:qtn, :512])
            r0 = q0 + jt * 128
            nc.scalar.dma_start(y_d[r0:r0 + qtn, d0:d0 + 512], yo[:qtn, :])

        def mk_ot_pair(ot_ps, pt, heads, sp, qn, last):
            def fn():
                ensure_v(sp)
                ensure_v(sp + 1)
                for h in heads:
                    nc.tensor.matmul(
                        ot_ps[h][:, :qn],
                        v_aug[:, sp:sp + 2, h, :],
                        pt[h][:, sp:sp + 2, :qn],
                        start=(sp == 0), stop=last, perf_mode=DR)
            return fn

        def mk_ot_single(ot_ps, pt, heads, s, qn):
            def fn():
                ensure_v(s)
                for h in heads:
                    nc.tensor.matmul(
                        ot_ps[h][:, :qn],
                        v_aug[:, s, h, 0:68],
                        pt[h][:, s, :qn],
                        start=(s == 0), stop=True)
            return fn

        def mk_chain_all(ot_ps, rb, qn):
            # 1/(Z+CADD) for all 4 heads in one chain: Z rows -> one SBUF
            # [4, qz] tile -> one [qp, 16] DMA transpose-pack (wide DVE
            # reciprocal; single-lane [1, qn] is ~15x slower) -> one DRAM
            # bounce -> 4 partition-broadcast DMAs
            def fn():
                qz = (qn + 15) // 16 * 16
                qp = qz // 4
                zall = z_pool.tile([1, 4, 512], f32, tag="zrow", name="zall")
                if qz > qn:
                    nc.vector.memset(zall[0:1, :, qn:qz], 1.0)
                for h in range(HC):
                    nc.vector.tensor_scalar(
                        out=zall[0:1, h, :qn], in0=ot_ps[h][64:65, :qn],
                        scalar1=CADD, scalar2=None, op0=mybir.AluOpType.add)
                # sync queue (HWDGE, ~0.6us/issue; gpsimd SWDGE is ~1us);
                # chain hops are emitted before this chunk's y DMAs, and y
                # rides the scalar queue anyway
                zq = z_pool.tile([128, 16], f32, tag="zq", name="zq")
                nc.sync.dma_start(zq[:qp, :], zall[0:1, :, :qz])
                nc.vector.reciprocal(zq[:qp, :], zq[:qp, :])
                zd = zscr.tile([4, 512], f32, tag="zd", name="zd")
                nc.sync.dma_start(zd[0:4, :qz], zq[:qp, :])
                for h in range(HC):
                    r0 = (h % 2) * 64
                    dqb = nc.sync if h % 2 == 0 else nc.scalar
                    dqb.dma_start(rb[r0:r0 + 64, h // 2, :qn],
                                  zd[h:h + 1, :qn].to_broadcast((64, qn)))
            return fn

        def mk_stt(ot_ps, rb, osball, p, h, qn):
            def fn():
                r0 = (h % 2) * 64
                nc.vector.scalar_tensor_tensor(
                    out=osball[r0:r0 + 64, p, :qn],
                    in0=ot_ps[h][0:64, :qn],
                    scalar=1.0,
                    in1=rb[r0:r0 + 64, p, :qn],
                    op0=mybir.AluOpType.mult,
                    op1=mybir.AluOpType.mult)
            return fn

        for ci, (q0, qn) in enumerate(qord0):
            # all 4 heads at once: score matmuls rotate PE row-tile
            # positions 0/32/64/96, which is what lets LDWEIGHTS pipeline
            # (measured 62 ns/score at 4-deep rotation vs 425 ns blocked)
            tail = nst * qn <= 512
            pt = {}
            for h in range(HC):
                pt[h] = pt_pool.tile([128, nst, 512], fp8, tag="pt",
                                     name="pt")
            if not tail:
                osball = osb_pool.tile([128, 2, 512], fp8, tag="osb",
                                       name="osball")
                y_osb[(q0, qn)] = osball
                ot_ps = {}
                for h in range(HC):
                    ot_ps[h] = otp.tile([68, 512], f32, tag="ot",
                                        name="otps")
                rb = z_pool.tile([128, 2, 512], f32, tag="rb", name="rb")
            if tail:
                # everything rides the fast-recycling "st" ring: no OT-ring
                # WAR on the previous chunk's chain, no on-device r at all
                run_due()
                for h in range(HC):
                    ensure_q(h // 2, q0)
                    ensure_km(h // 2, nst - 1)
                drain(len(aux))
                for s in range(nst):
                    ensure_v(s)
                zrows = z_pool.tile([1, 4, 64], f32, tag="ztl", name="zrows")
                otu = z_pool.tile([128, 512], fp8, tag="otu", name="otu")
                for h in range(HC):
                    st = stp.tile([128, nst, qn], f32, tag="st", name="st")
                    for s in range(nst):
                        nc.tensor.matmul(
                            st[:, s, :qn],
                            kt[32 * h:32 * h + 32, :,
                               s * 128:(s + 1) * 128],
                            qt[32 * h:32 * h + 32, :, q0:q0 + qn],
                            start=True, stop=True, perf_mode=DR,
                            tile_position=(32 * h, 0))
                    nc.scalar.activation(
                        pt[h][:, :, :qn], st[:, :, :qn],
                        mybir.ActivationFunctionType.Exp,
                        scale=ESCALE, bias=ebias[:, :])
                    tick()
                for h in range(HC):
                    r0 = (h % 2) * 64
                    otps = stp.tile([128, 512], f32, tag="st", name="otps")
                    for sp in range(0, nst - 1, 2):
                        nc.tensor.matmul(
                            otps[0:68, :qn],
                            v_aug[:, sp:sp + 2, h, :],
                            pt[h][:, sp:sp + 2, :qn],
                            start=(sp == 0), stop=(sp + 2 == nst),
                            perf_mode=DR)
                    if nst % 2 == 1:
                        nc.tensor.matmul(
                            otps[0:68, :qn],
                            v_aug[:, nst - 1, h, 0:68],
                            pt[h][:, nst - 1, :qn],
                            start=(nst == 1), stop=True)
                    nc.vector.tensor_copy(zrows[0:1, h, :qn],
                                          otps[64:65, :qn])
                    # unnormalized OT can exceed fp8 range: store OT/64
                    nc.vector.tensor_scalar(
                        out=otu[r0:r0 + 64, :qn], in0=otps[0:64, :qn],
                        scalar1=1.0 / 64.0, scalar2=None,
                        op0=mybir.AluOpType.mult)
                    for di, d0 in enumerate((0, 512)):
                        yh = stp.tile([128, 512], f32, tag="st", name="yh")
                        nc.tensor.matmul(
                            yh[:qn, :512],
                            otu[r0:r0 + 64, :qn],
                            wo_sb[r0:r0 + 64, h // 2, d0:d0 + 512],
                            start=True, stop=True)
                        yho = yo_pool.tile([128, 512], bf16, tag="yo",
                                           name="yho")
                        nc.scalar.copy(yho[:qn, :], yh[:qn, :512])
                        nc.scalar.dma_start(
                            yt_d[h, 0:qn, d0:d0 + 512], yho[:qn, :])
                nc.sync.dma_start(zt_d[0:1, :, :], zrows[0:1, :, :])
                run_due(flush=True)
                continue
            else:
                for s in range(nst):
                    run_due()
                    drain(1)
                    sts = {}
                    for h in range(HC):
                        if h % 2 == 0:
                            ensure_q(h // 2, q0)
                            ensure_km(h // 2, s)
                        sts[h] = stp.tile([128, 512], f32, tag="st",
                                          name="st")
                        nc.tensor.matmul(
                            sts[h][:, :qn],
                            kt[32 * h:32 * h + 32, :,
                               s * 128:(s + 1) * 128],
                            qt[32 * h:32 * h + 32, :, q0:q0 + qn],
                            start=True, stop=True, perf_mode=DR,
                            tile_position=(32 * h, 0))
                    for h in range(HC):
                        nc.scalar.activation(
                            pt[h][:, s, :qn], sts[h][:, :qn],
                            mybir.ActivationFunctionType.Exp,
                            scale=ESCALE, bias=ebias[:, :])
                    if s % 2 == 1:
                        after(2, mk_ot_pair(ot_ps, pt, range(HC), s - 1, qn,
                                            s + 1 == nst))
                    tick()
            # emission-order constraints (in-order queues + pool-slot WAR
            # deps bind at emission): single-OT < chain < STT, and STT(c)
            # before chunk c+1's first OT; all Y(c) before STT(c+2)
            if tail:
                for j in range(0, nst - 1, 2):
                    after(1, mk_ot_pair(ot_ps, pt, range(HC), j, qn,
                                        j + 2 == nst))
            if nst % 2 == 1:
                after(1, mk_ot_single(ot_ps, pt, range(HC), nst - 1, qn))
            after(1, mk_chain_all(ot_ps, rb, qn))
            for h in range(HC):
                after(2, mk_stt(ot_ps, rb, osball, h // 2, h, qn))
            last_chunk = ci == len(qord0) - 1
            for jt in range((qn + 127) // 128):
                for hi, d0 in enumerate((0, 512)):
                    after(4 + jt + hi,
                          lambda a=q0, b=qn, j=jt, d=d0, L=last_chunk:
                          y_job(a, b, j, d, L))
        drain(len(aux))
        while vq:
            proj_v(vq.pop(0))
        run_due(flush=True)
    nc.compile()
    return nc


_nc_cache: dict = {}


def _get_nc(SAQ, SAK):
    key = (SAQ, SAK)
    if key not in _nc_cache:
        _nc_cache[key] = _build(SAQ, SAK)
    return _nc_cache[key]


def _score_perm():
    """W column permutation (within each head-group's 256 cols) for the
    DR-32 layout: psum partitions [0:64] of the m-half are the j=0 subtile
    (dims 0..31 of heads 2m, 2m+1), [64:128] the j=1 subtile (dims 32..63),
    so head h sits at qt[32h:32h+32, j, :] with dim d = j*32 + p%32."""
    perm = np.empty(2 * 128, np.int64)
    for m in range(2):
        for p in range(128):
            h = 2 * m + (p % 64) // 32
            d = (p // 64) * 32 + (p % 32)
            perm[m * 128 + p] = h * 64 + d
    return perm


def _shuf(a):
    """[D, C] -> [128, D//128, C] host-contiguous fp8 (partition-major)."""
    Dd, C = a.shape
    return np.ascontiguousarray(
        a.reshape(Dd // 128, 128, C).transpose(1, 0, 2)).astype(npf8)


def _reference_fallback(x, gate, Wq, bq, Wk, bk, Wv, bv, Wo, bo):
    g = gate.astype(x.dtype)[..., None]
    q = (x @ Wq + bq) * g
    k = (x @ Wk + bk) * g
    v = (x @ Wv + bv) * g

    def split(t):
        return t.reshape(B, S, H, DH).transpose(0, 2, 1, 3)

    q, k, v = split(q), split(k), split(v)
    sc = np.einsum('bhqd,bhkd->bhqk', q, k) / np.float32(math.sqrt(DH))
    sc = sc - sc.max(axis=-1, keepdims=True)
    e = np.exp(sc)
    attn = e / e.sum(axis=-1, keepdims=True)
    out = np.einsum('bhqk,bhkd->bhqd', attn, v)
    out = out.transpose(0, 2, 1, 3).reshape(B, S, D)
    out = out @ Wo + bo
    return (x * (1.0 - g) + out * g).astype(np.float32)


def kernel(x, gate, Wq, bq, Wk, bk, Wv, bv, Wo, bo, _profile=None):
    x = np.asarray(x, np.float32)
    gate = np.asarray(gate)
    args = dict(x=x, gate=gate, Wq=np.asarray(Wq, np.float32),
                bq=np.asarray(bq, np.float32), Wk=np.asarray(Wk, np.float32),
                bk=np.asarray(bk, np.float32), Wv=np.asarray(Wv, np.float32),
                bv=np.asarray(bv, np.float32), Wo=np.asarray(Wo, np.float32),
                bo=np.asarray(bo, np.float32))

    idxs = [np.nonzero(gate[b])[0] for b in range(B)]
    n_act = [len(i) for i in idxs]
    # the compaction trick needs zero q/k/v biases and at least one active
    # and one inactive token per batch; otherwise fall back to exact numpy
    if (any(np.abs(args[k]).max() > 0 for k in ("bq", "bk", "bv"))
            or min(n_act) == 0 or max(n_act) == S):
        return _reference_fallback(**args)

    SAQ = max(n_act)
    SAK = ((SAQ + 127) // 128) * 128
    perm = _score_perm()

    in_maps = []
    for b in range(B):
        xa = np.zeros((SAK, D), np.float32)
        xa[:n_act[b]] = x[b, idxs[b]]
        xT = _shuf(np.ascontiguousarray(xa.T))
        for g in range(GROUPS):
            cs = slice(g * DC, (g + 1) * DC)
            in_maps.append({
                "xT": xT,
                "wq": _shuf(args["Wq"][:, cs][:, perm] * WSCALE),
                "wk": _shuf(args["Wk"][:, cs][:, perm] * WSCALE),
                "wv": _shuf(args["Wv"][:, cs] * WSCALE),
                "wo": np.ascontiguousarray(
                    (args["Wo"][cs, :] * WSCALE).reshape(2, 128, D)
                    .transpose(1, 0, 2)).astype(npf8),
            })

    nc = _get_nc(SAQ, SAK)
    kw = dict(_profile) if _profile else {}
    kw.pop("result", None)
    res = run_bass_kernel_spmd(nc, in_maps, core_ids=list(range(NCORES)), **kw)
    if _profile is not None:
        _profile["result"] = res

    out = x.copy()
    inv = 1.0 / (WSCALE * WSCALE)
    nst = SAK // 128
    qch = []
    o = 0
    while o < SAQ:
        qch.append((o, min(512, SAQ - o)))
        o += 512
    tq0, tqn = qch[-1]
    has_tail = nst * tqn <= 512
    cadd = float(S - SAK) / 16.0
    for b in range(B):
        Y = np.zeros((SAQ, D), np.float32)
        for g in range(GROUPS):
            r = res.results[b * GROUPS + g]
            yv = r["y"].astype(np.float32)
            if has_tail:
                yv[tq0:] = 0.0   # device never writes tail rows of y
            Y += yv
            if has_tail:
                # narrow-tail rows: per-head unnormalized Y + raw Z; apply
                # the softmax denominator here
                yt = np.asarray(r["yt"], np.float32)    # [4, 64, D]
                zt = np.asarray(r["zt"], np.float32)[0]  # [4, 64]
                rr = 64.0 / (zt[:, :tqn] + cadd)         # [4, tqn]
                Y[tq0:tq0 + tqn] += np.einsum(
                    'hqd,hq->qd', yt[:, :tqn], rr)
        out[b, idxs[b]] = Y[:n_act[b]] * inv + args["bo"]
    return out
